# revision 32
# baseline (speedup 1.0000x reference)
"""GQA Trainium2 Bass kernel, v2 (overlap-optimized).

Sharding: 8 cores = 2 batches x 4 KV groups. Per core (b, g):
kT = Wk_g^T X_k^T [128, S]; qT per head [128, 512] per s1-chunk; V^T ->
PE-transposed v tiles [s2, hd]; scores^T = kT_t^T qT (s2-major), exp on
ACT -> bf16 weights; softmax denominators via ones-matmul on PE; AV
accumulation [hd, s1]; normalization via DVE mul with a Pool-engine
partition_broadcast of the reciprocal; Wo row-shard partial product
[S, E]. Host sums the 4 group partials per batch + bo.

Overlap design:
- bf16 X and weights from host (halves DMA traffic + SBUF)
- host pre-arranges weights into [128, ...] layouts: one DMA each
- emission software-pipelines: V-proj/scores bootstrap interleaving;
  steady-state steps emit next-chunk Q-proj, next-step scores (t-wise
  interleaved), current ones+AV, and previous-chunk Wo matmuls
- ACT runs ONLY Exp (evictions on DVE) to avoid act-table reloads
- PSUM banks: pp=3 (proj/transpose/Wo), scores=2, AV=2, ones=1 = 8

build(loop_trips=N) wraps the body in a hardware For_i loop (weights
hoisted) for stable device-time measurement.
"""
import sys
sys.path.insert(0, '/opt/trn_rl_repo')
from contextlib import ExitStack

import numpy as np

import concourse.bass as bass
import concourse.tile as tile
from concourse import bacc, mybir
from concourse.masks import make_identity

E, NH, G, HD = 2048, 16, 4, 128
KV = E // G            # 512
B, S = 2, 2048
MQ = (NH // G) * HD    # 512 q columns per group
P = 128
SC = S // 512          # 4 s1-chunks of 512
ECN = E // P           # 16 contraction chunks
NT = S // P            # 16 s2 tiles
H = NH // G            # 4 heads per core
N_CORES = 8
F32 = mybir.dt.float32
F32R = mybir.dt.float32r
BF16 = mybir.dt.bfloat16
SCALE = float(HD) ** -0.5
EXPF = mybir.ActivationFunctionType.Exp
FP8 = mybir.dt.float8e4
DR = mybir.MatmulPerfMode.DoubleRow

X_DT = BF16
W_DT = BF16
USE_DR = False
DR_COPIES = False
DR_BLOCK = False


def build(loop_trips=None):
    nc = bacc.Bacc("TRN2", target_bir_lowering=False, debug=False,
                   num_devices=N_CORES)

    xq_d = nc.dram_tensor("xq", [E, S], X_DT, kind="ExternalInput").ap()
    xk_d = nc.dram_tensor("xk", [E, S], X_DT, kind="ExternalInput").ap()
    xv_d = nc.dram_tensor("xv", [E, S], X_DT, kind="ExternalInput").ap()
    wq_d = nc.dram_tensor("wq", [P, ECN * MQ], W_DT, kind="ExternalInput").ap()
    wk_d = nc.dram_tensor("wk", [P, ECN * HD], W_DT, kind="ExternalInput").ap()
    wv_d = nc.dram_tensor("wv", [P, ECN * HD], W_DT, kind="ExternalInput").ap()
    wo_d = nc.dram_tensor("wo", [P, H * 4 * 512], W_DT, kind="ExternalInput").ap()
    bq_d = nc.dram_tensor("bq", [P, H], F32, kind="ExternalInput").ap()
    bk_d = nc.dram_tensor("bk", [P, 1], F32, kind="ExternalInput").ap()
    bv_d = nc.dram_tensor("bv", [P, 1], F32, kind="ExternalInput").ap()
    out_d = nc.dram_tensor("out", [S, E], F32, kind="ExternalOutput").ap()

    with tile.TileContext(nc) as tc:
        with ExitStack() as ctx:
            # SBUF pools
            smp = ctx.enter_context(tc.tile_pool(name="smp", bufs=1))
            wts = ctx.enter_context(tc.tile_pool(name="wts", bufs=1))
            xkp = ctx.enter_context(tc.tile_pool(name="xkp", bufs=3))
            xqp = ctx.enter_context(tc.tile_pool(name="xqp", bufs=32))
            ktp = ctx.enter_context(tc.tile_pool(name="ktp", bufs=2))
            qtp = ctx.enter_context(tc.tile_pool(name="qtp", bufs=8))
            vsp = ctx.enter_context(tc.tile_pool(name="vsp", bufs=2))
            vtp = ctx.enter_context(tc.tile_pool(name="vtp", bufs=20))
            ewp = ctx.enter_context(tc.tile_pool(name="ewp", bufs=34))
            atp = ctx.enter_context(tc.tile_pool(name="atp", bufs=8))
            smu = ctx.enter_context(tc.tile_pool(name="smu", bufs=16))
            rcp = ctx.enter_context(tc.tile_pool(name="rcp", bufs=2))
            rbp = ctx.enter_context(tc.tile_pool(name="rbp", bufs=2))
            obp = ctx.enter_context(tc.tile_pool(name="obp", bufs=2))
            # PSUM pools: 3 + 2 + 2 + 1 = 8 banks
            pp = ctx.enter_context(tc.tile_pool(name="pp", bufs=3, space="PSUM"))
            psc = ctx.enter_context(tc.tile_pool(name="psc", bufs=2, space="PSUM"))
            pav = ctx.enter_context(tc.tile_pool(name="pav", bufs=2, space="PSUM"))
            pon = ctx.enter_context(tc.tile_pool(name="pon", bufs=1, space="PSUM"))

            # constants
            ident_f = smp.tile([P, P], F32, tag="ident_f")
            make_identity(nc, ident_f[:])
            ident = smp.tile([P, P], F32R, tag="ident")
            nc.vector.tensor_copy(ident[:], ident_f[:])
            ones_t = smp.tile([P, 1], BF16, tag="ones")
            nc.vector.memset(ones_t[:], 1.0)
            negb = smp.tile([P, 1], F32, tag="negb")
            nc.vector.memset(negb[:], -1.0)

            bq_t = smp.tile([P, H], F32, tag="bq")
            nc.sync.dma_start(bq_t[:], bq_d[:, :])
            bk_t = smp.tile([P, 1], F32, tag="bk")
            nc.sync.dma_start(bk_t[:], bk_d[:, :])
            bv_t = smp.tile([P, 1], F32, tag="bv")
            nc.sync.dma_start(bv_t[:], bv_d[:, :])

            wt = {}

            def load_w(key, dram, cols):
                t = wts.tile([P, cols], W_DT, tag=key, name=f"w_{key}")
                nc.sync.dma_start(t[:], dram[:, :])
                wt[key] = t

            def body(weights_inline):
                if weights_inline:
                    load_w("wk", wk_d, ECN * HD)

                # ---- K projection half 0 (s2 tiles 0..7) ----
                kT = ktp.tile([P, S], BF16, tag="kT")

                def emit_khalf(half):
                    ps_k = [pp.tile([P, 512], F32, tag="pp", name=f"ps_k{j}")
                            for j in range(2)]
                    for e in range(ECN):
                        xt = xkp.tile([P, 1024], X_DT, tag="xk")
                        nc.sync.dma_start(
                            xt[:], xk_d[e * P:(e + 1) * P,
                                        half * 1024:(half + 1) * 1024])
                        for j in range(2):
                            nc.tensor.matmul(
                                ps_k[j][:], wt["wk"][:, e * HD:(e + 1) * HD],
                                xt[:, j * 512:(j + 1) * 512],
                                start=(e == 0), stop=(e == ECN - 1))
                    for j in range(2):
                        off = half * 1024 + j * 512
                        nc.vector.tensor_add(
                            kT[:, off:off + 512], ps_k[j][:],
                            bk_t[:].broadcast_to([P, 512]))

                emit_khalf(0)

                # ---- Q chunk-0 X + weights ----
                if weights_inline:
                    load_w("wq", wq_d, ECN * MQ)
                xq_tiles = {c: [] for c in range(SC)}
                for e in range(ECN):
                    xt = xqp.tile([P, 512], X_DT, tag="xq")
                    nc.sync.dma_start(
                        xt[:], xq_d[e * P:(e + 1) * P, 0:512])
                    xq_tiles[0].append(xt)

                def emit_qproj(c, h):
                    ps_q = pp.tile([P, 512], F32, tag="pp", name="ps_q")
                    for e in range(ECN):
                        nc.tensor.matmul(
                            ps_q[:],
                            wt["wq"][:, e * MQ + h * P: e * MQ + (h + 1) * P],
                            xq_tiles[c][e][:],
                            start=(e == 0), stop=(e == ECN - 1))
                    qt = qtp.tile([P, 512], BF16, tag="qt", name=f"qt{c}_{h}")
                    nc.vector.tensor_add(
                        qt[:], ps_q[:],
                        bq_t[:, h:h + 1].broadcast_to([P, 512]))
                    return qt

                ew_tiles = {}
                qt_tiles = {}

                def emit_score_tile(step, t):
                    c, h = divmod(step, H)
                    ps_s = psc.tile([P, 512], F32, tag="psc")
                    nc.tensor.matmul(ps_s[:], kT[:, t * P:(t + 1) * P],
                                     qt_tiles[(c, h)][:], start=True, stop=True)
                    ew = ewp.tile([P, 512], BF16, tag="ew")
                    nc.scalar.activation(ew[:], ps_s[:], EXPF, scale=SCALE,
                                         bias=negb[:])
                    ew_tiles.setdefault(step, []).append(ew)


                # Q(c0,h0) then first scores on kT half 0 while K half 1
                # and V still stream in.
                qt_tiles[(0, 0)] = emit_qproj(0, 0)
                for t in range(NT // 2):
                    emit_score_tile(0, t)

                emit_khalf(1)
                for h in range(1, H):
                    qt_tiles[(0, h)] = emit_qproj(0, h)
                for t in range(NT // 2, NT):
                    emit_score_tile(0, t)

                # ---- V projection ----
                if weights_inline:
                    load_w("wv", wv_d, ECN * HD)
                v_tiles = []
                for c in range(SC):
                    ps_v = pp.tile([P, 512], F32, tag="pp", name="ps_v")
                    for e in range(ECN):
                        xt = xkp.tile([P, 512], X_DT, tag="xk")
                        nc.sync.dma_start(
                            xt[:], xv_d[e * P:(e + 1) * P,
                                        c * 512:(c + 1) * 512])
                        nc.tensor.matmul(ps_v[:],
                                         wt["wv"][:, e * HD:(e + 1) * HD],
                                         xt[:], start=(e == 0),
                                         stop=(e == ECN - 1))
                    vsb = vsp.tile([P, 512], F32R, tag="vsb")
                    nc.vector.tensor_add(vsb[:], ps_v[:],
                                         bv_t[:].broadcast_to([P, 512]))
                    for t in range(4):
                        pst = pp.tile([P, 512], F32R, tag="pp", name="pst")
                        nc.tensor.transpose(pst[:, 0:P],
                                            vsb[:, t * P:(t + 1) * P],
                                            ident[:])
                        vt_ = vtp.tile([P, P], BF16, tag="vt")
                        nc.vector.tensor_copy(vt_[:], pst[:, 0:P])
                        v_tiles.append(vt_)

                # ---- remaining DMAs: wo, then xq c1..c3 ----
                if weights_inline:
                    load_w("wo", wo_d, H * 4 * 512)
                for c in range(1, SC):
                    for e in range(ECN):
                        xt = xqp.tile([P, 512], X_DT, tag="xq")
                        nc.sync.dma_start(
                            xt[:], xq_d[e * P:(e + 1) * P,
                                        c * 512:(c + 1) * 512])
                        xq_tiles[c].append(xt)

                attnT = {}

                def emit_wo(c, st):
                    ob = obp.tile([P, E], F32, tag="ob")
                    for ecx in range(4):
                        ps_o = pp.tile([P, 512], F32, tag="pp", name="ps_o")
                        for hh in range(H):
                            nc.tensor.matmul(
                                ps_o[:],
                                attnT[(c, hh)][:, st * P:(st + 1) * P],
                                wt["wo"][:, (hh * 4 + ecx) * 512:
                                         (hh * 4 + ecx + 1) * 512],
                                start=(hh == 0), stop=(hh == H - 1))
                        nc.vector.tensor_copy(
                            ob[:, ecx * 512:(ecx + 1) * 512], ps_o[:])
                    s1t = c * 4 + st
                    nc.gpsimd.dma_start(
                        out_d[s1t * P:(s1t + 1) * P, :], ob[:])

                # ---- steady-state attention ----
                for step in range(SC * H):
                    c, h = divmod(step, H)
                    if c < SC - 1:
                        qt_tiles[(c + 1, h)] = emit_qproj(c + 1, h)
                    cur = ew_tiles.pop(step)
                    ps_ones = pon.tile([1, 512], F32, tag="pon")
                    ps_av = pav.tile([P, 512], F32, tag="pav")

                    has_next = step + 1 < SC * H
                    pairs = []
                    for t in range(NT):
                        if has_next:
                            emit_score_tile(step + 1, t)
                        nc.tensor.matmul(ps_av[:], v_tiles[t][:], cur[t][:],
                                         start=(t == 0), stop=(t == NT - 1))
                        if t % 2 == 1:
                            sp_ = smu.tile([P, 512], BF16, tag="smu",
                                           name=f"sp{t}")
                            nc.gpsimd.tensor_add(sp_[:], cur[t - 1][:],
                                                 cur[t][:])
                            pairs.append(sp_)
                    # DVE tree over the 8 pair sums, then one ones-matmul
                    lvl = pairs
                    while len(lvl) > 1:
                        nxt_lvl = []
                        for a, b_ in zip(lvl[0::2], lvl[1::2]):
                            s_ = smu.tile([P, 512], BF16, tag="smu",
                                          name="tree")
                            nc.vector.tensor_add(s_[:], a[:], b_[:])
                            nxt_lvl.append(s_)
                        lvl = nxt_lvl
                    nc.tensor.matmul(ps_ones[:], ones_t[:], lvl[0][:],
                                     start=True, stop=True)
                    rcb = rbp.tile([P, 512], F32, tag="rcb")
                    rc = rcp.tile([1, 512], F32, tag="rc")
                    nc.vector.reciprocal(rc[:], ps_ones[:])
                    nc.gpsimd.partition_broadcast(rcb[:], rc[:])
                    at = atp.tile([P, 512], BF16, tag="at")
                    nc.vector.tensor_mul(at[:], ps_av[:], rcb[:])
                    attnT[(c, h)] = at
                    if c >= 1:
                        emit_wo(c - 1, h)
                for st in range(4):
                    emit_wo(SC - 1, st)

            if loop_trips is None:
                body(weights_inline=True)
            else:
                load_w("wk", wk_d, ECN * HD)
                load_w("wq", wq_d, ECN * MQ)
                load_w("wv", wv_d, ECN * HD)
                load_w("wo", wo_d, H * 4 * 512)
                with tc.For_i(0, loop_trips, 1):
                    body(weights_inline=False)

    nc.compile()
    return nc


_CACHE = {}


def _get_nc():
    if "nc" not in _CACHE:
        _CACHE["nc"] = build()
    return _CACHE["nc"]


def _to_bf16(a):
    """f32 ndarray -> bf16 (round-to-nearest-even) as ml_dtypes.bfloat16."""
    import ml_dtypes
    a = np.ascontiguousarray(a, np.float32)
    u = a.view(np.uint32)
    r = ((u >> 16) & 1) + 0x7FFF
    return ((u + r) >> 16).astype(np.uint16).view(ml_dtypes.bfloat16)


def make_in_maps(query, key_in, value, Wq, bq, Wk, bk, Wv, bv, Wo, bo):
    f32 = np.float32
    xT = {}
    for b in range(B):
        xT[b] = (
            np.ascontiguousarray(_to_bf16(np.asarray(query[b], f32)).T),
            np.ascontiguousarray(_to_bf16(np.asarray(key_in[b], f32)).T),
            np.ascontiguousarray(_to_bf16(np.asarray(value[b], f32)).T),
        )
    Wq, Wk, Wv, Wo = (np.asarray(a, f32) for a in (Wq, Wk, Wv, Wo))
    bq, bk, bv = (np.asarray(a, f32) for a in (bq, bk, bv))

    wq_p, wk_p, wv_p, wo_p, bq_p, bk_p, bv_p = {}, {}, {}, {}, {}, {}, {}
    for g in range(G):
        wq_g = _to_bf16(Wq[:, g * MQ:(g + 1) * MQ])        # [E, 512]
        wq_p[g] = np.ascontiguousarray(
            wq_g.reshape(ECN, P, MQ).transpose(1, 0, 2).reshape(P, ECN * MQ))
        wk_g = _to_bf16(Wk[:, g * HD:(g + 1) * HD])        # [E, 128]
        wk_p[g] = np.ascontiguousarray(
            wk_g.reshape(ECN, P, HD).transpose(1, 0, 2).reshape(P, ECN * HD))
        wv_g = _to_bf16(Wv[:, g * HD:(g + 1) * HD])
        wv_p[g] = np.ascontiguousarray(
            wv_g.reshape(ECN, P, HD).transpose(1, 0, 2).reshape(P, ECN * HD))
        wo_g = _to_bf16(Wo[g * MQ:(g + 1) * MQ, :])        # [512, E]
        wo_p[g] = np.ascontiguousarray(
            wo_g.reshape(H, P, 4, 512).transpose(1, 0, 2, 3).reshape(P, -1))
        bq_p[g] = np.ascontiguousarray(
            bq[g * MQ:(g + 1) * MQ].reshape(H, P).T)       # [128, 4]
        bk_p[g] = np.ascontiguousarray(
            bk[g * HD:(g + 1) * HD].reshape(P, 1))
        bv_p[g] = np.ascontiguousarray(
            bv[g * HD:(g + 1) * HD].reshape(P, 1))

    in_maps = []
    for core in range(N_CORES):
        b, g = divmod(core, G)
        xq, xk, xv = xT[b]
        in_maps.append({
            "xq": xq, "xk": xk, "xv": xv,
            "wq": wq_p[g], "wk": wk_p[g], "wv": wv_p[g], "wo": wo_p[g],
            "bq": bq_p[g], "bk": bk_p[g], "bv": bv_p[g],
        })
    return in_maps


def assemble(results, bo):
    bo = np.asarray(bo, np.float32)
    out = np.empty((B, S, E), np.float32)
    for b in range(B):
        acc = results[b * G]["out"].astype(np.float32)
        for g in range(1, G):
            acc = acc + results[b * G + g]["out"]
        out[b] = acc + bo[None, :]
    return out


def kernel(query, key_in, value, Wq, bq, Wk, bk, Wv, bv, Wo, bo):
    from concourse.bass_utils import run_bass_kernel_spmd
    nc = _get_nc()
    in_maps = make_in_maps(query, key_in, value, Wq, bq, Wk, bk, Wv, bv, Wo, bo)
    res = run_bass_kernel_spmd(nc, in_maps, core_ids=list(range(N_CORES)))
    return assemble(res.results, bo)


# revision 34
# speedup vs baseline: 1.0057x; 1.0057x over previous
"""GQA Trainium2 Bass kernel, v2 (overlap-optimized).

Sharding: 8 cores = 2 batches x 4 KV groups. Per core (b, g):
kT = Wk_g^T X_k^T [128, S]; qT per head [128, 512] per s1-chunk; V^T ->
PE-transposed v tiles [s2, hd]; scores^T = kT_t^T qT (s2-major), exp on
ACT -> bf16 weights; softmax denominators via ones-matmul on PE; AV
accumulation [hd, s1]; normalization via DVE mul with a Pool-engine
partition_broadcast of the reciprocal; Wo row-shard partial product
[S, E]. Host sums the 4 group partials per batch + bo.

Overlap design:
- bf16 X and weights from host (halves DMA traffic + SBUF)
- host pre-arranges weights into [128, ...] layouts: one DMA each
- emission software-pipelines: V-proj/scores bootstrap interleaving;
  steady-state steps emit next-chunk Q-proj, next-step scores (t-wise
  interleaved), current ones+AV, and previous-chunk Wo matmuls
- ACT runs ONLY Exp (evictions on DVE) to avoid act-table reloads
- PSUM banks: pp=3 (proj/transpose/Wo), scores=2, AV=2, ones=1 = 8

build(loop_trips=N) wraps the body in a hardware For_i loop (weights
hoisted) for stable device-time measurement.
"""
import sys
sys.path.insert(0, '/opt/trn_rl_repo')
from contextlib import ExitStack

import numpy as np

import concourse.bass as bass
import concourse.tile as tile
from concourse import bacc, mybir
from concourse.masks import make_identity

E, NH, G, HD = 2048, 16, 4, 128
KV = E // G            # 512
B, S = 2, 2048
MQ = (NH // G) * HD    # 512 q columns per group
P = 128
SC = S // 512          # 4 s1-chunks of 512
ECN = E // P           # 16 contraction chunks
NT = S // P            # 16 s2 tiles
H = NH // G            # 4 heads per core
N_CORES = 8
F32 = mybir.dt.float32
F32R = mybir.dt.float32r
BF16 = mybir.dt.bfloat16
SCALE = float(HD) ** -0.5
EXPF = mybir.ActivationFunctionType.Exp
FP8 = mybir.dt.float8e4
DR = mybir.MatmulPerfMode.DoubleRow

X_DT = BF16
W_DT = BF16
USE_DR = False
DR_COPIES = False
DR_BLOCK = False


def build(loop_trips=None):
    nc = bacc.Bacc("TRN2", target_bir_lowering=False, debug=False,
                   num_devices=N_CORES)

    xq_d = nc.dram_tensor("xq", [E, S], X_DT, kind="ExternalInput").ap()
    xk_d = nc.dram_tensor("xk", [E, S], X_DT, kind="ExternalInput").ap()
    xv_d = nc.dram_tensor("xv", [E, S], X_DT, kind="ExternalInput").ap()
    wq_d = nc.dram_tensor("wq", [P, ECN * MQ], W_DT, kind="ExternalInput").ap()
    wk_d = nc.dram_tensor("wk", [P, ECN * HD], W_DT, kind="ExternalInput").ap()
    wv_d = nc.dram_tensor("wv", [P, ECN * HD], W_DT, kind="ExternalInput").ap()
    wo_d = nc.dram_tensor("wo", [P, H * 4 * 512], W_DT, kind="ExternalInput").ap()
    bq_d = nc.dram_tensor("bq", [P, H], F32, kind="ExternalInput").ap()
    bk_d = nc.dram_tensor("bk", [P, 1], F32, kind="ExternalInput").ap()
    bv_d = nc.dram_tensor("bv", [P, 1], F32, kind="ExternalInput").ap()
    out_d = nc.dram_tensor("out", [S, E], F32, kind="ExternalOutput").ap()

    with tile.TileContext(nc) as tc:
        with ExitStack() as ctx:
            # SBUF pools
            smp = ctx.enter_context(tc.tile_pool(name="smp", bufs=1))
            wts = ctx.enter_context(tc.tile_pool(name="wts", bufs=1))
            xkp = ctx.enter_context(tc.tile_pool(name="xkp", bufs=3))
            xqp = ctx.enter_context(tc.tile_pool(name="xqp", bufs=32))
            ktp = ctx.enter_context(tc.tile_pool(name="ktp", bufs=2))
            qtp = ctx.enter_context(tc.tile_pool(name="qtp", bufs=8))
            vsp = ctx.enter_context(tc.tile_pool(name="vsp", bufs=2))
            vtp = ctx.enter_context(tc.tile_pool(name="vtp", bufs=20))
            ewp = ctx.enter_context(tc.tile_pool(name="ewp", bufs=34))
            atp = ctx.enter_context(tc.tile_pool(name="atp", bufs=8))
            smu = ctx.enter_context(tc.tile_pool(name="smu", bufs=16))
            rcp = ctx.enter_context(tc.tile_pool(name="rcp", bufs=2))
            rbp = ctx.enter_context(tc.tile_pool(name="rbp", bufs=2))
            obp = ctx.enter_context(tc.tile_pool(name="obp", bufs=2))
            # PSUM pools: 3 + 2 + 2 + 1 = 8 banks
            pp = ctx.enter_context(tc.tile_pool(name="pp", bufs=3, space="PSUM"))
            psc = ctx.enter_context(tc.tile_pool(name="psc", bufs=2, space="PSUM"))
            pav = ctx.enter_context(tc.tile_pool(name="pav", bufs=2, space="PSUM"))
            pon = ctx.enter_context(tc.tile_pool(name="pon", bufs=1, space="PSUM"))

            # constants
            ident_f = smp.tile([P, P], F32, tag="ident_f")
            make_identity(nc, ident_f[:])
            ident = smp.tile([P, P], F32R, tag="ident")
            nc.vector.tensor_copy(ident[:], ident_f[:])
            ones_t = smp.tile([P, 1], BF16, tag="ones")
            nc.vector.memset(ones_t[:], 1.0)
            negb = smp.tile([P, 1], F32, tag="negb")
            nc.vector.memset(negb[:], -1.0)

            bq_t = smp.tile([P, H], F32, tag="bq")
            nc.sync.dma_start(bq_t[:], bq_d[:, :])
            bk_t = smp.tile([P, 1], F32, tag="bk")
            nc.sync.dma_start(bk_t[:], bk_d[:, :])
            bv_t = smp.tile([P, 1], F32, tag="bv")
            nc.sync.dma_start(bv_t[:], bv_d[:, :])

            wt = {}

            def load_w(key, dram, cols):
                t = wts.tile([P, cols], W_DT, tag=key, name=f"w_{key}")
                nc.sync.dma_start(t[:], dram[:, :])
                wt[key] = t

            def body(weights_inline):
                if weights_inline:
                    load_w("wk", wk_d, ECN * HD)

                # ---- K projection half 0 (s2 tiles 0..7) ----
                kT = ktp.tile([P, S], BF16, tag="kT")

                def emit_khalf(half):
                    ps_k = [pp.tile([P, 512], F32, tag="pp", name=f"ps_k{j}")
                            for j in range(2)]
                    for e in range(ECN):
                        xt = xkp.tile([P, 1024], X_DT, tag="xk")
                        nc.sync.dma_start(
                            xt[:], xk_d[e * P:(e + 1) * P,
                                        half * 1024:(half + 1) * 1024])
                        for j in range(2):
                            nc.tensor.matmul(
                                ps_k[j][:], wt["wk"][:, e * HD:(e + 1) * HD],
                                xt[:, j * 512:(j + 1) * 512],
                                start=(e == 0), stop=(e == ECN - 1))
                    for j in range(2):
                        off = half * 1024 + j * 512
                        nc.vector.tensor_add(
                            kT[:, off:off + 512], ps_k[j][:],
                            bk_t[:].broadcast_to([P, 512]))

                emit_khalf(0)

                # ---- Q chunk-0 X + weights ----
                if weights_inline:
                    load_w("wq", wq_d, ECN * MQ)
                xq_tiles = {c: [] for c in range(SC)}
                for e in range(ECN):
                    xt = xqp.tile([P, 512], X_DT, tag="xq")
                    nc.sync.dma_start(
                        xt[:], xq_d[e * P:(e + 1) * P, 0:512])
                    xq_tiles[0].append(xt)

                def emit_qproj(c, h):
                    ps_q = pp.tile([P, 512], F32, tag="pp", name="ps_q")
                    for e in range(ECN):
                        nc.tensor.matmul(
                            ps_q[:],
                            wt["wq"][:, e * MQ + h * P: e * MQ + (h + 1) * P],
                            xq_tiles[c][e][:],
                            start=(e == 0), stop=(e == ECN - 1))
                    qt = qtp.tile([P, 512], BF16, tag="qt", name=f"qt{c}_{h}")
                    nc.vector.tensor_add(
                        qt[:], ps_q[:],
                        bq_t[:, h:h + 1].broadcast_to([P, 512]))
                    return qt

                ew_tiles = {}
                qt_tiles = {}

                def emit_score_tile(step, t):
                    c, h = divmod(step, H)
                    ps_s = psc.tile([P, 512], F32, tag="psc")
                    nc.tensor.matmul(ps_s[:], kT[:, t * P:(t + 1) * P],
                                     qt_tiles[(c, h)][:], start=True, stop=True)
                    ew = ewp.tile([P, 512], BF16, tag="ew")
                    nc.scalar.activation(ew[:], ps_s[:], EXPF, scale=SCALE,
                                         bias=negb[:])
                    ew_tiles.setdefault(step, []).append(ew)


                # Q(c0,h0) then first scores on kT half 0 while K half 1
                # and V still stream in.
                qt_tiles[(0, 0)] = emit_qproj(0, 0)
                for t in range(NT // 2):
                    emit_score_tile(0, t)

                emit_khalf(1)
                for h in range(1, H):
                    qt_tiles[(0, h)] = emit_qproj(0, h)
                for t in range(NT // 2, NT):
                    emit_score_tile(0, t)

                # ---- V projection ----
                if weights_inline:
                    load_w("wv", wv_d, ECN * HD)
                v_tiles = []
                for c in range(SC):
                    ps_v = pp.tile([P, 512], F32, tag="pp", name="ps_v")
                    for e in range(ECN):
                        xt = xkp.tile([P, 512], X_DT, tag="xk")
                        nc.sync.dma_start(
                            xt[:], xv_d[e * P:(e + 1) * P,
                                        c * 512:(c + 1) * 512])
                        nc.tensor.matmul(ps_v[:],
                                         wt["wv"][:, e * HD:(e + 1) * HD],
                                         xt[:], start=(e == 0),
                                         stop=(e == ECN - 1))
                    vsb = vsp.tile([P, 512], F32R, tag="vsb")
                    nc.vector.tensor_add(vsb[:], ps_v[:],
                                         bv_t[:].broadcast_to([P, 512]))
                    for t in range(4):
                        pst = pp.tile([P, 512], F32R, tag="pp", name="pst")
                        nc.tensor.transpose(pst[:, 0:P],
                                            vsb[:, t * P:(t + 1) * P],
                                            ident[:])
                        vt_ = vtp.tile([P, P], BF16, tag="vt")
                        nc.vector.tensor_copy(vt_[:], pst[:, 0:P])
                        v_tiles.append(vt_)

                # ---- remaining DMAs: wo, then xq c1..c3 ----
                if weights_inline:
                    load_w("wo", wo_d, H * 4 * 512)
                for c in range(1, SC):
                    for e in range(ECN):
                        xt = xqp.tile([P, 512], X_DT, tag="xq")
                        nc.sync.dma_start(
                            xt[:], xq_d[e * P:(e + 1) * P,
                                        c * 512:(c + 1) * 512])
                        xq_tiles[c].append(xt)

                attnT = {}

                def emit_wo(c, st):
                    ob = obp.tile([P, E], F32, tag="ob")
                    for ecx in range(4):
                        ps_o = pp.tile([P, 512], F32, tag="pp", name="ps_o")
                        for hh in range(H):
                            nc.tensor.matmul(
                                ps_o[:],
                                attnT[(c, hh)][:, st * P:(st + 1) * P],
                                wt["wo"][:, (hh * 4 + ecx) * 512:
                                         (hh * 4 + ecx + 1) * 512],
                                start=(hh == 0), stop=(hh == H - 1))
                        nc.vector.tensor_copy(
                            ob[:, ecx * 512:(ecx + 1) * 512], ps_o[:])
                    s1t = c * 4 + st
                    nc.gpsimd.dma_start(
                        out_d[s1t * P:(s1t + 1) * P, :], ob[:])

                # ---- steady-state attention ----
                for step in range(SC * H):
                    c, h = divmod(step, H)
                    if c < SC - 1:
                        qt_tiles[(c + 1, h)] = emit_qproj(c + 1, h)
                    cur = ew_tiles.pop(step)
                    ps_ones = pon.tile([1, 512], F32, tag="pon")
                    ps_av = pav.tile([P, 512], F32, tag="pav")

                    has_next = step + 1 < SC * H
                    pairs = []
                    for t in range(NT):
                        if has_next:
                            emit_score_tile(step + 1, t)
                        nc.tensor.matmul(ps_av[:], v_tiles[t][:], cur[t][:],
                                         start=(t == 0), stop=(t == NT - 1))
                        if t % 2 == 1:
                            sp_ = smu.tile([P, 512], BF16, tag="smu",
                                           name=f"sp{t}")
                            nc.gpsimd.tensor_add(sp_[:], cur[t - 1][:],
                                                 cur[t][:])
                            pairs.append(sp_)
                    # DVE tree over the 8 pair sums, then one ones-matmul
                    lvl = pairs
                    while len(lvl) > 1:
                        nxt_lvl = []
                        for a, b_ in zip(lvl[0::2], lvl[1::2]):
                            s_ = smu.tile([P, 512], BF16, tag="smu",
                                          name="tree")
                            nc.vector.tensor_add(s_[:], a[:], b_[:])
                            nxt_lvl.append(s_)
                        lvl = nxt_lvl
                    nc.tensor.matmul(ps_ones[:], ones_t[:], lvl[0][:],
                                     start=True, stop=True)
                    rcb = rbp.tile([P, 512], F32, tag="rcb")
                    rc = rcp.tile([1, 512], F32, tag="rc")
                    nc.vector.reciprocal(rc[:], ps_ones[:])
                    nc.gpsimd.partition_broadcast(rcb[:], rc[:])
                    at = atp.tile([P, 512], BF16, tag="at")
                    nc.vector.tensor_mul(at[:], ps_av[:], rcb[:])
                    attnT[(c, h)] = at
                    if c >= 1:
                        emit_wo(c - 1, h)
                for st in range(4):
                    emit_wo(SC - 1, st)

            if loop_trips is None:
                body(weights_inline=True)
            else:
                load_w("wk", wk_d, ECN * HD)
                load_w("wq", wq_d, ECN * MQ)
                load_w("wv", wv_d, ECN * HD)
                load_w("wo", wo_d, H * 4 * 512)
                with tc.For_i(0, loop_trips, 1):
                    body(weights_inline=False)

    nc.compile()
    return nc


_CACHE = {}


def _get_nc():
    if "nc" not in _CACHE:
        _CACHE["nc"] = build()
    return _CACHE["nc"]


def _to_bf16(a):
    """f32 ndarray -> bf16 (round-to-nearest-even) as ml_dtypes.bfloat16."""
    import ml_dtypes
    a = np.ascontiguousarray(a, np.float32)
    u = a.view(np.uint32)
    r = ((u >> 16) & 1) + 0x7FFF
    return ((u + r) >> 16).astype(np.uint16).view(ml_dtypes.bfloat16)


def make_in_maps(query, key_in, value, Wq, bq, Wk, bk, Wv, bv, Wo, bo):
    f32 = np.float32
    xT = {}
    for b in range(B):
        xT[b] = (
            np.ascontiguousarray(_to_bf16(np.asarray(query[b], f32)).T),
            np.ascontiguousarray(_to_bf16(np.asarray(key_in[b], f32)).T),
            np.ascontiguousarray(_to_bf16(np.asarray(value[b], f32)).T),
        )
    Wq, Wk, Wv, Wo = (np.asarray(a, f32) for a in (Wq, Wk, Wv, Wo))
    bq, bk, bv = (np.asarray(a, f32) for a in (bq, bk, bv))

    wq_p, wk_p, wv_p, wo_p, bq_p, bk_p, bv_p = {}, {}, {}, {}, {}, {}, {}
    for g in range(G):
        wq_g = _to_bf16(Wq[:, g * MQ:(g + 1) * MQ])        # [E, 512]
        wq_p[g] = np.ascontiguousarray(
            wq_g.reshape(ECN, P, MQ).transpose(1, 0, 2).reshape(P, ECN * MQ))
        wk_g = _to_bf16(Wk[:, g * HD:(g + 1) * HD])        # [E, 128]
        wk_p[g] = np.ascontiguousarray(
            wk_g.reshape(ECN, P, HD).transpose(1, 0, 2).reshape(P, ECN * HD))
        wv_g = _to_bf16(Wv[:, g * HD:(g + 1) * HD])
        wv_p[g] = np.ascontiguousarray(
            wv_g.reshape(ECN, P, HD).transpose(1, 0, 2).reshape(P, ECN * HD))
        wo_g = _to_bf16(Wo[g * MQ:(g + 1) * MQ, :])        # [512, E]
        wo_p[g] = np.ascontiguousarray(
            wo_g.reshape(H, P, 4, 512).transpose(1, 0, 2, 3).reshape(P, -1))
        bq_p[g] = np.ascontiguousarray(
            bq[g * MQ:(g + 1) * MQ].reshape(H, P).T)       # [128, 4]
        bk_p[g] = np.ascontiguousarray(
            bk[g * HD:(g + 1) * HD].reshape(P, 1))
        bv_p[g] = np.ascontiguousarray(
            bv[g * HD:(g + 1) * HD].reshape(P, 1))

    in_maps = []
    for core in range(N_CORES):
        b, g = divmod(core, G)
        xq, xk, xv = xT[b]
        in_maps.append({
            "xq": xq, "xk": xk, "xv": xv,
            "wq": wq_p[g], "wk": wk_p[g], "wv": wv_p[g], "wo": wo_p[g],
            "bq": bq_p[g], "bk": bk_p[g], "bv": bv_p[g],
        })
    return in_maps


def assemble(results, bo):
    bo = np.asarray(bo, np.float32)
    out = np.empty((B, S, E), np.float32)
    for b in range(B):
        acc = results[b * G]["out"].astype(np.float32)
        for g in range(1, G):
            acc = acc + results[b * G + g]["out"]
        out[b] = acc + bo[None, :]
    return out


def kernel(query, key_in, value, Wq, bq, Wk, bk, Wv, bv, Wo, bo):
    from concourse.bass_utils import run_bass_kernel_spmd
    nc = _get_nc()
    in_maps = make_in_maps(query, key_in, value, Wq, bq, Wk, bk, Wv, bv, Wo, bo)
    res = run_bass_kernel_spmd(nc, in_maps, core_ids=list(range(N_CORES)))
    return assemble(res.results, bo)
